# revision 1
# baseline (speedup 1.0000x reference)
"""AttnBlock (GroupNorm + single-head self-attention + residual) on 8 TRN2 cores.

Sharding: core c = 2*b + h handles batch b, query-half h. Each core computes
GroupNorm + K/V over the full image of its batch (stats need the full batch;
K/V compute is duplicated across the pair of cores, avoiding any collectives)
and Q/attention/output for its 2048 of the 4096 pixels. The per-core input
image is column-permuted so the owned half is always columns [0, 2048) —
GroupNorm stats and the softmax sum over keys are permutation-invariant, so
the result is exact.

Exact algebra folds: bk is dropped (softmax over keys is invariant to a
per-query logit shift); bv is folded into bo' = wo @ bv + bo on the host;
the softmax 1/den normalization commutes with the output projection (it
scales along the free dim) and is applied at output evacuation.

Layouts on chip (partition dim first):
  hn, k, q: [C, N] as 4 tiles [128, N];  vT: 32 tiles [j=128, C]
  scores computed transposed [j, i] so softmax reductions over keys j land on
  the PE (ones-matmul denominator) and no attention transpose is ever needed.
"""

import numpy as np
import ml_dtypes

B, C, HW = 4, 512, 4096
NQ = HW // 2          # queries per core
GROUPS = 32
EPS = 1e-5
N_CORES = 8
CI = C // 128         # 4 chunks of 128 channels
IB = NQ // 512        # 4 i-blocks of 512 queries
JB = HW // 512        # 8 j-blocks of 512 keys
JC = HW // 128        # 32 key chunks of 128
SCALE = float(C) ** -0.5

_cache = {}


def _build():
    import concourse.tile as tile
    from concourse import bacc, mybir

    F32 = mybir.dt.float32
    BF16 = mybir.dt.bfloat16
    AF = mybir.ActivationFunctionType
    ALU = mybir.AluOpType

    nc = bacc.Bacc("TRN2", target_bir_lowering=False, debug=False,
                   num_devices=N_CORES)

    xf_ap = nc.dram_tensor("xf", [C, HW], F32, kind="ExternalInput").ap()
    w_aps = {
        w: nc.dram_tensor(w, [C, C], BF16, kind="ExternalInput").ap()
        for w in ("wqT", "wkT", "wvT", "woT")
    }
    bq_ap = nc.dram_tensor("bq2", [C, 1], F32, kind="ExternalInput").ap()
    bo_ap = nc.dram_tensor("bo2", [C, 1], F32, kind="ExternalInput").ap()
    ga_ap = nc.dram_tensor("ga", [C, 1], F32, kind="ExternalInput").ap()
    gb_ap = nc.dram_tensor("gb", [C, 1], F32, kind="ExternalInput").ap()
    selBB_ap = nc.dram_tensor("selBB", [128, 128], F32, kind="ExternalInput").ap()
    out_ap = nc.dram_tensor("out", [C, NQ], F32, kind="ExternalOutput").ap()

    H2 = HW // 2  # DMA half width

    with tile.TileContext(nc) as tc:
        with (
            tc.tile_pool(name="wsb", bufs=1) as wsb,
            tc.tile_pool(name="small", bufs=1) as small,
            tc.tile_pool(name="hn", bufs=1) as hn_pool,
            tc.tile_pool(name="attn", bufs=6) as attn_pool,
            tc.tile_pool(name="aosb", bufs=1) as aosb_pool,
            tc.tile_pool(name="rb", bufs=2) as rb_pool,
            tc.tile_pool(name="xres", bufs=2) as xres_pool,
            tc.tile_pool(name="oevac", bufs=3) as oevac,
        ):
            hn = [hn_pool.tile([128, HW], BF16, tag=f"hn{t}", name=f"hn{t}")
                  for t in range(CI)]

            # ================= Phase 1: GroupNorm =================
            with (
                tc.tile_pool(name="xin", bufs=1) as xin,
                tc.tile_pool(name="scrap", bufs=2) as scrap_pool,
                tc.tile_pool(name="gn_ps", bufs=1, space="PSUM") as gn_ps,
            ):
                # x loads first: everything else queues behind them
                Q4 = HW // 4
                x_t = []
                for t in range(CI):
                    xt = xin.tile([128, HW], F32, tag=f"x{t}", name=f"x{t}")
                    for hh in range(4):
                        nc.sync.dma_start(
                            xt[:, hh * Q4:(hh + 1) * Q4],
                            xf_ap[t * 128:(t + 1) * 128, hh * Q4:(hh + 1) * Q4])
                    x_t.append(xt)

                # small constants (gpsimd queue, won't block x)
                selBB = small.tile([128, 128], F32, tag="selBB")
                nc.gpsimd.dma_start(selBB[:], selBB_ap[:])
                ones_b = small.tile([128, 1], BF16, tag="ones_b")
                nc.vector.memset(ones_b[:], 1.0)
                ones_f = small.tile([128, 128], F32, tag="ones_f")
                nc.vector.memset(ones_f[:], 1.0)
                recb = small.tile([128, 512], F32, tag="recb")
                nc.vector.memset(recb[:], 0.0)
                eps_t = small.tile([128, 4], F32, tag="eps")
                nc.vector.memset(eps_t[:], EPS)
                ga4 = small.tile([128, CI], F32, tag="ga4")
                gb4 = small.tile([128, CI], F32, tag="gb4")
                bq4 = small.tile([128, CI], F32, tag="bq4")
                bo4 = small.tile([128, CI], F32, tag="bo4")
                for t in range(CI):
                    nc.gpsimd.dma_start(ga4[:, t:t + 1], ga_ap[t * 128:(t + 1) * 128, :])
                    nc.gpsimd.dma_start(gb4[:, t:t + 1], gb_ap[t * 128:(t + 1) * 128, :])
                    nc.gpsimd.dma_start(bq4[:, t:t + 1], bq_ap[t * 128:(t + 1) * 128, :])
                    nc.gpsimd.dma_start(bo4[:, t:t + 1], bo_ap[t * 128:(t + 1) * 128, :])

                # weights (sync queue, after x)
                w_sb = {}
                for w in ("wqT", "wkT", "wvT", "woT"):
                    w_sb[w] = []
                    for t in range(CI):
                        tt = wsb.tile([128, C], BF16, tag=f"{w}{t}", name=f"{w}{t}")
                        nc.sync.dma_start(tt[:], w_aps[w][t * 128:(t + 1) * 128, :])
                        w_sb[w].append(tt)

                # stats per quarter: DVE row-sum, ACT square+accum
                # col layout: 4t+qq -> sum, 16+4t+qq -> sumsq
                stats = small.tile([128, 8 * CI], F32, tag="stats")
                for t in range(CI):
                    for qq in range(4):
                        sl = x_t[t][:, qq * Q4:(qq + 1) * Q4]
                        nc.vector.reduce_sum(
                            stats[:, 4 * t + qq:4 * t + qq + 1],
                            sl, axis=mybir.AxisListType.X)
                        scr = scrap_pool.tile([128, Q4], BF16, tag="scrap")
                        nc.scalar.activation(
                            scr[:], sl, AF.Square,
                            accum_out=stats[:, 16 + 4 * t + qq:17 + 4 * t + qq])

                # group-merge across partitions (replicated per-partition)
                G = gn_ps.tile([128, 8 * CI], F32, tag="G")
                nc.tensor.matmul(G[:], selBB[:], stats[:], start=True, stop=True)

                # selBB carries 1/(16*HW), so G is already mean-scaled
                Gs = small.tile([128, 8 * CI], F32, tag="Gs")
                nc.vector.tensor_copy(Gs[:], G[:])
                # pairwise-add quarters twice: 32 cols -> 16 -> 8 (mean 0:4, E[x^2] 4:8)
                p16 = small.tile([128, 4 * CI], F32, tag="p16")
                nc.vector.tensor_tensor(p16[:], Gs[:, 0:32:2], Gs[:, 1:32:2],
                                        op=ALU.add)
                mean8 = small.tile([128, 2 * CI], F32, tag="mean8")
                nc.vector.tensor_tensor(mean8[:], p16[:, 0:16:2], p16[:, 1:16:2],
                                        op=ALU.add)
                mean4 = mean8[:, 0:CI]
                ex24 = mean8[:, CI:2 * CI]
                m24 = small.tile([128, CI], F32, tag="m24")
                nc.vector.tensor_tensor(m24[:], mean4, mean4, op=ALU.mult)
                var4 = small.tile([128, CI], F32, tag="var4")
                nc.vector.tensor_tensor(var4[:], ex24, m24[:], op=ALU.subtract)
                sd4 = small.tile([128, CI], F32, tag="sd4")
                nc.scalar.activation(sd4[:], var4[:], AF.Sqrt, bias=eps_t[:, 0:1])
                rstd4 = small.tile([128, CI], F32, tag="rstd4")
                nc.vector.reciprocal(rstd4[:], sd4[:])
                a4 = small.tile([128, CI], F32, tag="a4")
                nc.vector.tensor_tensor(a4[:], rstd4[:], ga4[:], op=ALU.mult)
                am4 = small.tile([128, CI], F32, tag="am4")
                nc.vector.tensor_tensor(am4[:], mean4, a4[:], op=ALU.mult)
                b4 = small.tile([128, CI], F32, tag="b4")
                nc.vector.tensor_tensor(b4[:], gb4[:], am4[:], op=ALU.subtract)

                # normalize: DVE takes first half, ACT second half
                for t in range(CI):
                    nc.vector.tensor_scalar(hn[t][:, 0:H2], x_t[t][:, 0:H2],
                                            a4[:, t:t + 1], b4[:, t:t + 1],
                                            ALU.mult, ALU.add)
                    nc.scalar.activation(hn[t][:, H2:HW], x_t[t][:, H2:HW],
                                         AF.Identity, bias=b4[:, t:t + 1],
                                         scale=a4[:, t:t + 1])

            # ================= Phase 2: projections =================
            _kqv_cm = tc.tile_pool(name="kqv", bufs=1)
            kqv = _kqv_cm.__enter__()
            k_sb = [kqv.tile([128, HW], BF16, tag=f"k{t}", name=f"k{t}")
                    for t in range(CI)]
            q_sb = [kqv.tile([128, NQ], BF16, tag=f"q{t}", name=f"q{t}")
                    for t in range(CI)]
            vT_sb = [kqv.tile([128, C], BF16, tag=f"vT{j}", name=f"vT{j}")
                     for j in range(JC)]

            with tc.tile_pool(name="proj_ps", bufs=8, space="PSUM") as proj_ps:
                # k = wkT.T @ hn: per co, 8 jb-banks accumulate with ci outer
                # so each weight chunk is loaded once per (co, ci).
                for co in range(CI):
                    pss = [proj_ps.tile([128, 512], F32, tag="proj",
                                        name=f"kps{co}_{jb}") for jb in range(JB)]
                    for ci in range(CI):
                        for jb in range(JB):
                            nc.tensor.matmul(
                                pss[jb][:],
                                w_sb["wkT"][ci][:, co * 128:(co + 1) * 128],
                                hn[ci][:, jb * 512:(jb + 1) * 512],
                                start=(ci == 0), stop=(ci == CI - 1))
                    for jb in range(JB):
                        nc.vector.tensor_copy(k_sb[co][:, jb * 512:(jb + 1) * 512],
                                              pss[jb][:])
                # q = wqT.T @ hn[:, :NQ] + bq (4 ib-banks, ci outer)
                for co in range(CI):
                    pss = [proj_ps.tile([128, 512], F32, tag="proj",
                                        name=f"qps{co}_{ib}") for ib in range(IB)]
                    for ci in range(CI):
                        for ib in range(IB):
                            nc.tensor.matmul(
                                pss[ib][:],
                                w_sb["wqT"][ci][:, co * 128:(co + 1) * 128],
                                hn[ci][:, ib * 512:(ib + 1) * 512],
                                start=(ci == 0), stop=(ci == CI - 1))
                    for ib in range(IB):
                        nc.vector.tensor_scalar(
                            q_sb[co][:, ib * 512:(ib + 1) * 512], pss[ib][:],
                            bq4[:, co:co + 1], None, ALU.add)
                # vT[j, c] = hn_chunk.T @ wvT  (bias folded; evac on ACT)
                for jc in range(JC):
                    ps = proj_ps.tile([128, 512], F32, tag="proj",
                                      name=f"vps{jc}")
                    for ci in range(CI):
                        nc.tensor.matmul(
                            ps[:],
                            hn[ci][:, jc * 128:(jc + 1) * 128],
                            w_sb["wvT"][ci][:],
                            start=(ci == 0), stop=(ci == CI - 1))
                    nc.scalar.activation(vT_sb[jc][:], ps[:], AF.Copy)

            # ================= Phase 3: attention + output =================
            # Software-pipelined: scores+exp of step s+1 are emitted before
            # the attn@V of step s, so the PE never waits on the ACT exp.
            with (
                tc.tile_pool(name="sc_ps", bufs=2, space="PSUM") as sc_ps,
                tc.tile_pool(name="ao_ps", bufs=1, space="PSUM") as ao_ps,
                tc.tile_pool(name="y_ps", bufs=1, space="PSUM") as y_ps,
            ):
                seq = [(ib, jc) for ib in range(IB) for jc in range(JC)]
                at_tiles = {}
                ao_cur = {}
                xres_cur = {}

                def emit_scores(step):
                    ib, jc = seq[step]
                    sc = sc_ps.tile([128, 512], F32, tag="sc",
                                    name=f"sc{ib}_{jc}")
                    for ci in range(CI):
                        nc.tensor.matmul(
                            sc[:],
                            k_sb[ci][:, jc * 128:(jc + 1) * 128],
                            q_sb[ci][:, ib * 512:(ib + 1) * 512],
                            start=(ci == 0), stop=(ci == CI - 1))
                    at = attn_pool.tile([128, 512], BF16, tag="at",
                                        name=f"at{ib}_{jc}")
                    nc.scalar.activation(at[:], sc[:], AF.Exp, scale=SCALE)
                    at_tiles[step] = at

                emit_scores(0)
                emit_scores(1)
                for step, (ib, jc) in enumerate(seq):
                    if jc == 0:
                        # i-block entry: residual prefetch + fresh accumulators
                        xres_cur[ib] = []
                        for co in range(CI):
                            xr = xres_pool.tile([128, 512], F32, tag=f"xres{co}",
                                                name=f"xres{ib}_{co}")
                            nc.sync.dma_start(
                                xr[:],
                                xf_ap[co * 128:(co + 1) * 128,
                                      ib * 512:(ib + 1) * 512])
                            xr2 = xres_pool.tile([128, 512], F32, tag=f"xrb{co}",
                                                 name=f"xrb{ib}_{co}")
                            nc.scalar.activation(xr2[:], xr[:], AF.Identity,
                                                 bias=bo4[:, co:co + 1])
                            xres_cur[ib].append(xr2)
                        ao_cur[ib] = [ao_ps.tile([128, 512], F32, tag=f"ao{cc}",
                                                 name=f"ao{ib}_{cc}")
                                      for cc in range(CI)]
                        ao_cur[ib].append(ao_ps.tile([1, 512], F32, tag="den",
                                                     name=f"den{ib}"))
                    if step + 2 < len(seq):
                        emit_scores(step + 2)
                    at = at_tiles.pop(step)
                    ao = ao_cur[ib]
                    m_last = None
                    for cc in range(CI):
                        m_last = nc.tensor.matmul(
                            ao[cc][:],
                            vT_sb[jc][:, cc * 128:(cc + 1) * 128],
                            at[:],
                            start=(jc == 0), stop=(jc == JC - 1))
                    m_den = nc.tensor.matmul(ao[CI][:], ones_b[:], at[:],
                                             start=(jc == 0), stop=(jc == JC - 1))
                    tile.add_dep_helper(m_last.ins, m_den.ins, sync=False,
                                        reason="keep den after ao group")
                    if jc == JC - 1:
                        # post block: evac (DVE+ACT), recip/broadcast, o-proj
                        ao_n = []
                        for cc in range(CI):
                            an = aosb_pool.tile([128, 512], BF16, tag=f"aon{cc}",
                                                name=f"aon{ib}_{cc}")
                            if cc % 2 == 0:
                                nc.vector.tensor_copy(an[:], ao[cc][:])
                            else:
                                nc.scalar.activation(an[:], ao[cc][:], AF.Copy)
                            ao_n.append(an)
                        nc.vector.reciprocal_approx_fast(recb[0:1, :],
                                                         ao[CI][:])
                        rb_ps = y_ps.tile([128, 512], F32, tag="y",
                                          name=f"rbps{ib}")
                        nc.tensor.matmul(rb_ps[:], ones_f[:], recb[:],
                                         start=True, stop=True)
                        rb = rb_pool.tile([128, 512], F32, tag="rb",
                                          name=f"rb{ib}")
                        nc.scalar.activation(rb[:], rb_ps[:], AF.Copy)
                        for co in range(CI):
                            if ib == IB - 1:
                                yp = sc_ps.tile([128, 512], F32, tag="sc",
                                                name=f"y{ib}_{co}")
                            else:
                                yp = y_ps.tile([128, 512], F32, tag="y",
                                               name=f"y{ib}_{co}")
                            for cc in range(CI):
                                nc.tensor.matmul(
                                    yp[:],
                                    w_sb["woT"][cc][:, co * 128:(co + 1) * 128],
                                    ao_n[cc][:],
                                    start=(cc == 0), stop=(cc == CI - 1))
                            yn = oevac.tile([128, 512], F32, tag="yn")
                            nc.vector.tensor_tensor(yn[:], yp[:], rb[:],
                                                    op=ALU.mult)
                            ot = oevac.tile([128, 512], F32, tag="ot")
                            nc.vector.tensor_tensor(ot[:], yn[:],
                                                    xres_cur[ib][co][:],
                                                    op=ALU.add)
                            nc.sync.dma_start(
                                out_ap[co * 128:(co + 1) * 128,
                                       ib * 512:(ib + 1) * 512],
                                ot[:])
            _kqv_cm.__exit__(None, None, None)

    nc.compile()
    return nc


def _prep_inputs(x, norm_scale, norm_bias, wq, bq, wk, bk, wv, bv, wo, bo):
    bf16 = ml_dtypes.bfloat16
    f32 = np.float32
    x = np.asarray(x, f32).reshape(B, C, HW)
    common = {
        "wqT": np.ascontiguousarray(np.asarray(wq, f32).T).astype(bf16),
        "wkT": np.ascontiguousarray(np.asarray(wk, f32).T).astype(bf16),
        "wvT": np.ascontiguousarray(np.asarray(wv, f32).T).astype(bf16),
        "woT": np.ascontiguousarray(np.asarray(wo, f32).T).astype(bf16),
        "bq2": np.asarray(bq, f32).reshape(C, 1),
        "bo2": (np.asarray(wo, f32) @ np.asarray(bv, f32)
                + np.asarray(bo, f32)).reshape(C, 1),
        "ga": np.asarray(norm_scale, f32).reshape(C, 1),
        "gb": np.asarray(norm_bias, f32).reshape(C, 1),
        "selBB": (np.arange(128)[:, None] // 16
                  == np.arange(128)[None, :] // 16).astype(f32)
                 * np.float32(1.0 / (16 * HW)),
    }
    in_maps = []
    for c in range(N_CORES):
        b, h = divmod(c, 2)
        mine = x[b][:, h * NQ:(h + 1) * NQ]
        other = x[b][:, (1 - h) * NQ:(2 - h) * NQ]
        xf = np.ascontiguousarray(np.concatenate([mine, other], axis=1))
        in_maps.append({"xf": xf, **common})
    return in_maps


def _run(in_maps, **kwargs):
    from concourse.bass_utils import run_bass_kernel_spmd
    if "nc" not in _cache:
        _cache["nc"] = _build()
    return run_bass_kernel_spmd(_cache["nc"], in_maps,
                                core_ids=list(range(N_CORES)), **kwargs)


def kernel(x, norm_scale, norm_bias, wq, bq, wk, bk, wv, bv, wo, bo):
    in_maps = _prep_inputs(x, norm_scale, norm_bias, wq, bq, wk, bk, wv, bv,
                           wo, bo)
    res = _run(in_maps)
    out = np.empty((B, C, HW), np.float32)
    for c in range(N_CORES):
        b, h = divmod(c, 2)
        out[b][:, h * NQ:(h + 1) * NQ] = res.results[c]["out"]
    return out.reshape(B, C, 64, 64)



# revision 14
# speedup vs baseline: 1.4723x; 1.4723x over previous
"""AttnBlock (GroupNorm + single-head self-attention + residual) on 8 TRN2 cores.

Sharding: core c = 2*b + h handles batch b, query-half h. Each core computes
GroupNorm + K/V over the full image of its batch (stats need the full batch;
K/V compute is duplicated across the pair of cores, avoiding any collectives)
and Q/attention/output for its 2048 of the 4096 pixels. The per-core input
image is column-permuted so the owned half is always columns [0, 2048).

FP8 fast path: all five matmul stages (q/k/v proj, scores, attn@V, o-proj)
run in fp8e4 (e4m3) with MatmulPerfMode.DoubleRow, contracting 256 rows per
instruction at 2x the bf16 rate. Operands are laid out [128, 2, free] with
channel (or key) chunk pairs stacked on the middle dim. exp is computed as
exp(s*scale - 3) so the fp8 range (max 240) is never exceeded (max observed
logit ~7.2); the e^-3 factor cancels exactly in the softmax division.

Exact algebra folds: bk is dropped (softmax invariant to per-query logit
shift); bv is folded into bo' = wo @ bv + bo on the host; bo' enters the
o-proj PSUM via a K=1 matmul with the softmax denominator as rhs (so the
later 1/den normalization cancels it back to +bo'); the 1/den normalization
is applied at output evacuation (DVE mult with a gpsimd partition_broadcast
of the reciprocal row).

x is shipped to the device as bf16 (stats/normalize precision is plenty);
the residual is re-loaded as f32 off the critical path.
"""

import numpy as np
import ml_dtypes

B, C, HW = 4, 512, 4096
NQ = HW // 2          # queries per core
GROUPS = 32
EPS = 1e-5
N_CORES = 8
CI = C // 128         # 4 chunks of 128 channels
CP = CI // 2          # 2 chunk pairs
IB = NQ // 512        # 4 i-blocks of 512 queries
JB = HW // 512        # 8 j-blocks of 512 keys
JC = HW // 128        # 32 key chunks of 128
JP = JC // 2          # 16 key chunk pairs
SCALE = float(C) ** -0.5
EXP_OFF = 3.0         # exp(s*SCALE - 3): keeps fp8e4 in range, cancels in /den

_cache = {}


def _build():
    import concourse.tile as tile
    from concourse import bacc, mybir

    F32 = mybir.dt.float32
    BF16 = mybir.dt.bfloat16
    F8 = mybir.dt.float8e4
    AF = mybir.ActivationFunctionType
    ALU = mybir.AluOpType
    DR = mybir.MatmulPerfMode.DoubleRow

    nc = bacc.Bacc("TRN2", target_bir_lowering=False, debug=False,
                   num_devices=N_CORES)

    xbf_ap = nc.dram_tensor("xbf", [C, HW], BF16, kind="ExternalInput").ap()
    xres_ap = nc.dram_tensor("xres", [C, NQ], F32, kind="ExternalInput").ap()
    w_aps = {
        w: nc.dram_tensor(w, [2 * 128, 2 * C], F8, kind="ExternalInput").ap()
        for w in ("wqT", "wkT", "wvT", "woT")
    }
    bq_ap = nc.dram_tensor("bq2", [C, 1], F32, kind="ExternalInput").ap()
    bo_ap = nc.dram_tensor("bo2r", [1, C], BF16, kind="ExternalInput").ap()
    ga_ap = nc.dram_tensor("ga", [C, 1], F32, kind="ExternalInput").ap()
    gb_ap = nc.dram_tensor("gb", [C, 1], F32, kind="ExternalInput").ap()
    selBB_ap = nc.dram_tensor("selBB", [128, 128], F32, kind="ExternalInput").ap()
    ones2_ap = nc.dram_tensor("ones2", [128, 32], F8, kind="ExternalInput").ap()
    out_ap = nc.dram_tensor("out", [C, NQ], F32, kind="ExternalOutput").ap()

    H2 = HW // 2
    Q4 = HW // 4

    with tile.TileContext(nc) as tc:
        with (
            tc.tile_pool(name="wsb", bufs=1) as wsb,
            tc.tile_pool(name="small", bufs=1) as small,
            tc.tile_pool(name="hn", bufs=1) as hn_pool,
            tc.tile_pool(name="attn", bufs=3) as attn_pool,
            tc.tile_pool(name="aosb", bufs=2) as aosb_pool,
            tc.tile_pool(name="rb", bufs=2) as rb_pool,
            tc.tile_pool(name="xres", bufs=2) as xres_pool,
            tc.tile_pool(name="oevac", bufs=3) as oevac,
            tc.tile_pool(name="dsb", bufs=2) as dsb_pool,
        ):
            # hn in fp8, chunk-pair layout: hn_pair[p][:, m, :] = channels of
            # chunk 2p+m
            hn = [hn_pool.tile([128, 2, HW], F8, tag=f"hn{t}", name=f"hn{t}")
                  for t in range(CP)]

            # ================= Phase 1: GroupNorm =================
            with (
                tc.tile_pool(name="xin", bufs=1) as xin,
                tc.tile_pool(name="scrap", bufs=2) as scrap_pool,
                tc.tile_pool(name="gn_ps", bufs=1, space="PSUM") as gn_ps,
            ):
                # x (bf16) loads first: everything else queues behind them
                x_t = []
                for t in range(CI):
                    xt = xin.tile([128, HW], BF16, tag=f"x{t}", name=f"x{t}")
                    for hh in range(4):
                        nc.sync.dma_start(
                            xt[:, hh * Q4:(hh + 1) * Q4],
                            xbf_ap[t * 128:(t + 1) * 128, hh * Q4:(hh + 1) * Q4])
                    x_t.append(xt)

                # small constants (gpsimd queue, won't block x)
                selBB = small.tile([128, 128], F32, tag="selBB")
                nc.gpsimd.dma_start(selBB[:], selBB_ap[:])
                # [128, 2, 16]: DoubleRow lhsT outer free stride must be
                # 16B-aligned; only column 0 is consumed
                ones2 = small.tile([128, 2, 16], F8, tag="ones2")
                nc.gpsimd.dma_start(ones2[:], ones2_ap[:])
                ones_row = small.tile([1, 512], BF16, tag="ones_row")
                nc.vector.memset(ones_row[:], 1.0)
                bo2_sb = small.tile([1, C], BF16, tag="bo2_sb")
                nc.gpsimd.dma_start(bo2_sb[:], bo_ap[:])
                eps_t = small.tile([128, 4], F32, tag="eps")
                nc.vector.memset(eps_t[:], EPS)
                noff = small.tile([128, 1], F32, tag="noff")
                nc.vector.memset(noff[:], -EXP_OFF)
                ga4 = small.tile([128, CI], F32, tag="ga4")
                gb4 = small.tile([128, CI], F32, tag="gb4")
                bq4 = small.tile([128, CI], F32, tag="bq4")
                for t in range(CI):
                    nc.gpsimd.dma_start(ga4[:, t:t + 1], ga_ap[t * 128:(t + 1) * 128, :])
                    nc.gpsimd.dma_start(gb4[:, t:t + 1], gb_ap[t * 128:(t + 1) * 128, :])
                    nc.gpsimd.dma_start(bq4[:, t:t + 1], bq_ap[t * 128:(t + 1) * 128, :])

                # weights (sync queue, after x): fp8 pair layout
                w_sb = {}
                for w in ("wqT", "wkT", "wvT", "woT"):
                    w_sb[w] = []
                    for p in range(CP):
                        tt = wsb.tile([128, 2, C], F8, tag=f"{w}{p}",
                                      name=f"{w}{p}")
                        nc.sync.dma_start(tt[:],
                                          w_aps[w][p * 128:(p + 1) * 128, :])
                        w_sb[w].append(tt)

                # stats per quarter: DVE row-sum, ACT square+accum
                # col layout: 4t+qq -> sum, 16+4t+qq -> sumsq
                stats = small.tile([128, 8 * CI], F32, tag="stats")
                for t in range(CI):
                    for qq in range(4):
                        sl = x_t[t][:, qq * Q4:(qq + 1) * Q4]
                        nc.vector.reduce_sum(
                            stats[:, 4 * t + qq:4 * t + qq + 1],
                            sl, axis=mybir.AxisListType.X)
                        scr = scrap_pool.tile([128, Q4], BF16, tag="scrap")
                        nc.scalar.activation(
                            scr[:], sl, AF.Square,
                            accum_out=stats[:, 16 + 4 * t + qq:17 + 4 * t + qq])

                # group-merge across partitions (replicated per-partition)
                G = gn_ps.tile([128, 8 * CI], F32, tag="G")
                nc.tensor.matmul(G[:], selBB[:], stats[:], start=True, stop=True)

                # selBB carries 1/(16*HW), so G is already mean-scaled
                Gs = small.tile([128, 8 * CI], F32, tag="Gs")
                nc.vector.tensor_copy(Gs[:], G[:])
                # pairwise-add quarters twice: 32 cols -> 16 -> 8
                p16 = small.tile([128, 4 * CI], F32, tag="p16")
                nc.vector.tensor_tensor(p16[:], Gs[:, 0:32:2], Gs[:, 1:32:2],
                                        op=ALU.add)
                mean8 = small.tile([128, 2 * CI], F32, tag="mean8")
                nc.vector.tensor_tensor(mean8[:], p16[:, 0:16:2], p16[:, 1:16:2],
                                        op=ALU.add)
                mean4 = mean8[:, 0:CI]
                ex24 = mean8[:, CI:2 * CI]
                m24 = small.tile([128, CI], F32, tag="m24")
                nc.vector.tensor_tensor(m24[:], mean4, mean4, op=ALU.mult)
                var4 = small.tile([128, CI], F32, tag="var4")
                nc.vector.tensor_tensor(var4[:], ex24, m24[:], op=ALU.subtract)
                sd4 = small.tile([128, CI], F32, tag="sd4")
                nc.scalar.activation(sd4[:], var4[:], AF.Sqrt, bias=eps_t[:, 0:1])
                rstd4 = small.tile([128, CI], F32, tag="rstd4")
                nc.vector.reciprocal(rstd4[:], sd4[:])
                a4 = small.tile([128, CI], F32, tag="a4")
                nc.vector.tensor_tensor(a4[:], rstd4[:], ga4[:], op=ALU.mult)
                am4 = small.tile([128, CI], F32, tag="am4")
                nc.vector.tensor_tensor(am4[:], mean4, a4[:], op=ALU.mult)
                b4 = small.tile([128, CI], F32, tag="b4")
                nc.vector.tensor_tensor(b4[:], gb4[:], am4[:], op=ALU.subtract)

                # normalize into fp8 pair layout: DVE first half, ACT second
                for t in range(CI):
                    dst = hn[t // 2][:, (t % 2):(t % 2) + 1, :]
                    nc.vector.tensor_scalar(dst[:, :, 0:H2], x_t[t][:, 0:H2],
                                            a4[:, t:t + 1], b4[:, t:t + 1],
                                            ALU.mult, ALU.add)
                    nc.scalar.activation(dst[:, :, H2:HW], x_t[t][:, H2:HW],
                                         AF.Identity, bias=b4[:, t:t + 1],
                                         scale=a4[:, t:t + 1])

            # ================= Phase 2: projections (fp8 DoubleRow) ========
            _kqv_cm = tc.tile_pool(name="kqv", bufs=1)
            kqv = _kqv_cm.__enter__()
            k_pair = [kqv.tile([128, 2, HW], F8, tag=f"k{p}", name=f"k{p}")
                      for p in range(CP)]
            q_pair = [kqv.tile([128, 2, NQ], F8, tag=f"q{p}", name=f"q{p}")
                      for p in range(CP)]
            vT_pair = [kqv.tile([128, 2, C], F8, tag=f"vT{jp}", name=f"vT{jp}")
                       for jp in range(JP)]

            with tc.tile_pool(name="proj_ps", bufs=4, space="PSUM") as proj_ps:
                # k = wkT.T @ hn; per co, 4 psum pair-tiles, pair-evac'd
                for co in range(CI):
                    pps = [proj_ps.tile([128, 1024], F32, tag="proj",
                                        name=f"kps{co}_{g}") for g in range(4)]
                    for p in range(CP):
                        for jb in range(JB):
                            nc.tensor.matmul(
                                pps[jb // 2][:, (jb % 2) * 512:(jb % 2 + 1) * 512],
                                w_sb["wkT"][p][:, :, co * 128:(co + 1) * 128],
                                hn[p][:, :, jb * 512:(jb + 1) * 512],
                                start=(p == 0), stop=(p == CP - 1),
                                perf_mode=DR)
                    for g in range(4):
                        dst = k_pair[co // 2][:, (co % 2):(co % 2) + 1,
                                              g * 1024:(g + 1) * 1024]
                        if g % 2 == 0:
                            nc.vector.tensor_copy(dst, pps[g][:])
                        else:
                            nc.scalar.activation(dst, pps[g][:], AF.Copy)
                # q = wqT.T @ hn[:, :NQ] + bq
                for co in range(CI):
                    pps = [proj_ps.tile([128, 1024], F32, tag="proj",
                                        name=f"qps{co}_{g}") for g in range(2)]
                    for p in range(CP):
                        for ib in range(IB):
                            nc.tensor.matmul(
                                pps[ib // 2][:, (ib % 2) * 512:(ib % 2 + 1) * 512],
                                w_sb["wqT"][p][:, :, co * 128:(co + 1) * 128],
                                hn[p][:, :, ib * 512:(ib + 1) * 512],
                                start=(p == 0), stop=(p == CP - 1),
                                perf_mode=DR)
                    for g in range(2):
                        dst = q_pair[co // 2][:, (co % 2):(co % 2) + 1,
                                              g * 1024:(g + 1) * 1024]
                        nc.vector.tensor_scalar(dst, pps[g][:],
                                                bq4[:, co:co + 1], None,
                                                ALU.add)
                # vT[j, c] = hn_chunk.T @ wvT (bias folded into bo2)
                for jp in range(JP):
                    ps = proj_ps.tile([128, 1024], F32, tag="proj",
                                      name=f"vps{jp}")
                    for m in range(2):
                        jc = 2 * jp + m
                        for p in range(CP):
                            nc.tensor.matmul(
                                ps[:, m * 512:(m + 1) * 512],
                                hn[p][:, :, jc * 128:(jc + 1) * 128],
                                w_sb["wvT"][p][:],
                                start=(p == 0), stop=(p == CP - 1),
                                perf_mode=DR)
                    if jp % 2 == 0:
                        nc.vector.tensor_copy(vT_pair[jp][:], ps[:])
                    else:
                        nc.scalar.activation(vT_pair[jp][:], ps[:], AF.Copy)

            # ================= Phase 3: attention + output =================
            # Software-pipelined: scores+exp of pair-step s+1 are emitted
            # before the attn@V of step s, so the PE never waits on ACT exp.
            with (
                tc.tile_pool(name="sc_ps", bufs=3, space="PSUM") as sc_ps,
                tc.tile_pool(name="ao_ps", bufs=1, space="PSUM") as ao_ps,
            ):
                seq = [(ib, jp) for ib in range(IB) for jp in range(JP)]
                at_tiles = {}
                ao_cur = {}
                den_cur = {}
                xres_cur = {}

                def emit_scores(step):
                    ib, jp = seq[step]
                    at = attn_pool.tile([128, 2, 512], F8, tag="at",
                                        name=f"at{ib}_{jp}")
                    for m in range(2):
                        jc = 2 * jp + m
                        sc = sc_ps.tile([128, 512], F32, tag="sc",
                                        name=f"sc{ib}_{jc}")
                        for p in range(CP):
                            nc.tensor.matmul(
                                sc[:],
                                k_pair[p][:, :, jc * 128:(jc + 1) * 128],
                                q_pair[p][:, :, ib * 512:(ib + 1) * 512],
                                start=(p == 0), stop=(p == CP - 1),
                                perf_mode=DR)
                        nc.scalar.activation(at[:, m:m + 1, :], sc[:], AF.Exp,
                                             bias=noff[:], scale=SCALE)
                    at_tiles[step] = at

                emit_scores(0)
                emit_scores(1)
                for step, (ib, jp) in enumerate(seq):
                    if jp == 0:
                        # i-block entry: residual prefetch + fresh accumulators
                        xres_cur[ib] = []
                        for co in range(CI):
                            xr = xres_pool.tile([128, 512], F32, tag=f"xres{co}",
                                                name=f"xres{ib}_{co}")
                            nc.sync.dma_start(
                                xr[:],
                                xres_ap[co * 128:(co + 1) * 128,
                                        ib * 512:(ib + 1) * 512])
                            xres_cur[ib].append(xr)
                        ao_cur[ib] = [ao_ps.tile([128, 512], F32, tag=f"ao{cc}",
                                                 name=f"ao{ib}_{cc}")
                                      for cc in range(CI)]
                        den_cur[ib] = ao_ps.tile([1, 512], F32, tag="den",
                                                 name=f"den{ib}")
                    if step + 2 < len(seq):
                        emit_scores(step + 2)
                    at = at_tiles.pop(step)
                    ao = ao_cur[ib]
                    m_last = None
                    for cc in range(CI):
                        m_last = nc.tensor.matmul(
                            ao[cc][:],
                            vT_pair[jp][:, :, cc * 128:(cc + 1) * 128],
                            at[:],
                            start=(jp == 0), stop=(jp == JP - 1),
                            perf_mode=DR)
                    m_den = nc.tensor.matmul(den_cur[ib][:], ones2[:, :, 0:1],
                                             at[:],
                                             start=(jp == 0), stop=(jp == JP - 1),
                                             perf_mode=DR)
                    tile.add_dep_helper(m_last.ins, m_den.ins, sync=False,
                                        reason="keep den after ao group")
                    if jp == JP - 1:
                        # post block: recip+broadcast, normalized fp8 evac,
                        # o-proj, +bo2, residual
                        den = den_cur[ib]
                        recb = dsb_pool.tile([1, 512], F32, tag="recb",
                                             name=f"recb{ib}")
                        nc.vector.reciprocal_approx_fast(recb[:], den[:])
                        rb = rb_pool.tile([128, 512], F32, tag="rb",
                                          name=f"rb{ib}")
                        nc.gpsimd.partition_broadcast(rb[:], recb[:])
                        ao_n = []
                        for pp in range(CP):
                            an = aosb_pool.tile([128, 2, 512], F8, tag=f"aon{pp}",
                                                name=f"aon{ib}_{pp}")
                            for m in range(2):
                                cc = 2 * pp + m
                                # normalize during evac: fp8 only ever sees
                                # softmax-normalized values (|.| small)
                                nc.vector.tensor_tensor(an[:, m:m + 1, :],
                                                        ao[cc][:], rb[:],
                                                        op=ALU.mult)
                            ao_n.append(an)
                        for co in range(CI):
                            yp = sc_ps.tile([128, 512], F32, tag="sc",
                                            name=f"y{ib}_{co}")
                            for pp in range(CP):
                                nc.tensor.matmul(
                                    yp[:],
                                    w_sb["woT"][pp][:, :, co * 128:(co + 1) * 128],
                                    ao_n[pp][:],
                                    start=(pp == 0), stop=(pp == CP - 1),
                                    perf_mode=DR)
                            # + bo2 (broadcast along queries via ones row)
                            nc.tensor.matmul(
                                yp[:],
                                bo2_sb[:, co * 128:(co + 1) * 128],
                                ones_row[:],
                                start=False, stop=True, skip_group_check=True)
                            ot = oevac.tile([128, 512], F32, tag="ot")
                            nc.vector.tensor_tensor(ot[:], yp[:],
                                                    xres_cur[ib][co][:],
                                                    op=ALU.add)
                            nc.sync.dma_start(
                                out_ap[co * 128:(co + 1) * 128,
                                       ib * 512:(ib + 1) * 512],
                                ot[:])
            _kqv_cm.__exit__(None, None, None)

    nc.compile()
    return nc


def _prep_inputs(x, norm_scale, norm_bias, wq, bq, wk, bk, wv, bv, wo, bo):
    bf16 = ml_dtypes.bfloat16
    f8 = ml_dtypes.float8_e4m3
    f32 = np.float32
    x = np.asarray(x, f32).reshape(B, C, HW)

    def w8pair(w):
        # w: [out, in] f32 -> wT fp8 pair layout [2*128, 2*512]:
        # rows p*128..(p+1)*128 hold [128, 2, 512] = chunks (2p, 2p+1)
        wT = np.ascontiguousarray(np.asarray(w, f32).T)  # [cin, cout]
        wT8 = np.clip(wT, -240, 240).astype(f8)
        out = np.empty((2 * 128, 2 * C), f8)
        for p in range(CP):
            a = wT8[256 * p:256 * p + 128]          # [128, 512]
            b = wT8[256 * p + 128:256 * (p + 1)]    # [128, 512]
            out[p * 128:(p + 1) * 128] = np.stack([a, b], axis=1).reshape(128, 2 * C)
        return out

    common = {
        "wqT": w8pair(wq),
        "wkT": w8pair(wk),
        "wvT": w8pair(wv),
        "woT": w8pair(wo),
        "bq2": np.asarray(bq, f32).reshape(C, 1),
        "bo2r": (np.asarray(wo, f32) @ np.asarray(bv, f32)
                 + np.asarray(bo, f32)).reshape(1, C).astype(bf16),
        "ga": np.asarray(norm_scale, f32).reshape(C, 1),
        "gb": np.asarray(norm_bias, f32).reshape(C, 1),
        "selBB": (np.arange(128)[:, None] // 16
                  == np.arange(128)[None, :] // 16).astype(f32)
                 * np.float32(1.0 / (16 * HW)),
        "ones2": np.ones((128, 32), f8),
    }
    in_maps = []
    for c in range(N_CORES):
        b, h = divmod(c, 2)
        mine = x[b][:, h * NQ:(h + 1) * NQ]
        other = x[b][:, (1 - h) * NQ:(2 - h) * NQ]
        xf = np.ascontiguousarray(np.concatenate([mine, other], axis=1))
        in_maps.append({"xbf": xf.astype(bf16),
                        "xres": np.ascontiguousarray(mine),
                        **common})
    return in_maps


def _run(in_maps, **kwargs):
    from concourse.bass_utils import run_bass_kernel_spmd
    if "nc" not in _cache:
        _cache["nc"] = _build()
    return run_bass_kernel_spmd(_cache["nc"], in_maps,
                                core_ids=list(range(N_CORES)), **kwargs)


def kernel(x, norm_scale, norm_bias, wq, bq, wk, bk, wv, bv, wo, bo):
    in_maps = _prep_inputs(x, norm_scale, norm_bias, wq, bq, wk, bk, wv, bv,
                           wo, bo)
    res = _run(in_maps)
    out = np.empty((B, C, HW), np.float32)
    for c in range(N_CORES):
        b, h = divmod(c, 2)
        out[b][:, h * NQ:(h + 1) * NQ] = res.results[c]["out"]
    return out.reshape(B, C, 64, 64)


# revision 23
# speedup vs baseline: 1.5043x; 1.0218x over previous
"""AttnBlock (GroupNorm + single-head self-attention + residual) on 8 TRN2 cores.

Sharding: core c = 2*b + h handles batch b, query-half h. Each core computes
GroupNorm + K/V over the full image of its batch (stats need the full batch;
K/V compute is duplicated across the pair of cores, avoiding any collectives)
and Q/attention/output for its 2048 of the 4096 pixels. The per-core input
image is column-permuted so the owned half is always columns [0, 2048).

FP8 fast path: all five matmul stages (q/k/v proj, scores, attn@V, o-proj)
run in fp8e4 (e4m3) with MatmulPerfMode.DoubleRow, contracting 256 rows per
instruction at 2x the bf16 rate. Operands are laid out [128, 2, free] with
channel (or key) chunk pairs stacked on the middle dim. exp is computed as
exp(s*scale - 3) so the fp8 range (max 240) is never exceeded (max observed
logit ~7.2); the e^-3 factor cancels exactly in the softmax division.

Exact algebra folds: bk is dropped (softmax invariant to per-query logit
shift); bv is folded into bo' = wo @ bv + bo on the host; bo' enters the
o-proj PSUM via a K=1 matmul with the softmax denominator as rhs (so the
later 1/den normalization cancels it back to +bo'); the 1/den normalization
is applied at output evacuation (DVE mult with a gpsimd partition_broadcast
of the reciprocal row).

x is shipped to the device as bf16 (stats/normalize precision is plenty);
the residual is re-loaded as f32 off the critical path.
"""

import numpy as np
import ml_dtypes

B, C, HW = 4, 512, 4096
NQ = HW // 2          # queries per core
GROUPS = 32
EPS = 1e-5
N_CORES = 8
CI = C // 128         # 4 chunks of 128 channels
CP = CI // 2          # 2 chunk pairs
IB = NQ // 512        # 4 i-blocks of 512 queries
JB = HW // 512        # 8 j-blocks of 512 keys
JC = HW // 128        # 32 key chunks of 128
JP = JC // 2          # 16 key chunk pairs
SCALE = float(C) ** -0.5
EXP_OFF = 3.0         # exp(s*SCALE - 3): keeps fp8e4 in range, cancels in /den

_cache = {}


def _build():
    import concourse.tile as tile
    from concourse import bacc, mybir

    F32 = mybir.dt.float32
    BF16 = mybir.dt.bfloat16
    F8 = mybir.dt.float8e4
    AF = mybir.ActivationFunctionType
    ALU = mybir.AluOpType
    DR = mybir.MatmulPerfMode.DoubleRow

    nc = bacc.Bacc("TRN2", target_bir_lowering=False, debug=False,
                   num_devices=N_CORES)

    xbf_ap = nc.dram_tensor("xbf", [C, HW], BF16, kind="ExternalInput").ap()
    xres_ap = nc.dram_tensor("xres", [C, NQ], F32, kind="ExternalInput").ap()
    w_aps = {
        w: nc.dram_tensor(w, [2 * 128, 2 * C], F8, kind="ExternalInput").ap()
        for w in ("wqT", "wkT", "wvT", "woT")
    }
    bq_ap = nc.dram_tensor("bq2", [C, 1], F32, kind="ExternalInput").ap()
    bo_ap2 = nc.dram_tensor("bo2c", [C, 1], F32, kind="ExternalInput").ap()
    ga_ap = nc.dram_tensor("ga", [C, 1], F32, kind="ExternalInput").ap()
    gb_ap = nc.dram_tensor("gb", [C, 1], F32, kind="ExternalInput").ap()
    selBB_ap = nc.dram_tensor("selBB", [128, 128], F32, kind="ExternalInput").ap()
    ones2_ap = nc.dram_tensor("ones2", [128, 32], F8, kind="ExternalInput").ap()
    out_ap = nc.dram_tensor("out", [C, NQ], F32, kind="ExternalOutput").ap()

    H2 = HW // 2
    Q4 = HW // 4

    with tile.TileContext(nc) as tc:
        with (
            tc.tile_pool(name="wsb", bufs=1) as wsb,
            tc.tile_pool(name="small", bufs=1) as small,
            tc.tile_pool(name="hn", bufs=1) as hn_pool,
            tc.tile_pool(name="attn", bufs=3) as attn_pool,
            tc.tile_pool(name="aosb", bufs=2) as aosb_pool,
            tc.tile_pool(name="rb", bufs=2) as rb_pool,
            tc.tile_pool(name="xres", bufs=2) as xres_pool,
            tc.tile_pool(name="oevac", bufs=3) as oevac,
            tc.tile_pool(name="dsb", bufs=2) as dsb_pool,
        ):
            # hn in fp8, chunk-pair layout: hn_pair[p][:, m, :] = channels of
            # chunk 2p+m
            hn = [hn_pool.tile([128, 2, HW], F8, tag=f"hn{t}", name=f"hn{t}")
                  for t in range(CP)]

            # ================= Phase 1: GroupNorm =================
            with (
                tc.tile_pool(name="xin", bufs=1) as xin,
                tc.tile_pool(name="scrap", bufs=2) as scrap_pool,
                tc.tile_pool(name="gn_ps", bufs=1, space="PSUM") as gn_ps,
            ):
                # x (bf16) loads first, full rows (8KB descriptors: the DMA
                # engines are descriptor-overhead-bound below ~4KB)
                x_t = []
                for t in range(CI):
                    xt = xin.tile([128, HW], BF16, tag=f"x{t}", name=f"x{t}")
                    nc.sync.dma_start(xt[:], xbf_ap[t * 128:(t + 1) * 128, :])
                    x_t.append(xt)

                # small constants (gpsimd queue, won't block x)
                selBB = small.tile([128, 128], F32, tag="selBB")
                nc.gpsimd.dma_start(selBB[:], selBB_ap[:])
                # [128, 2, 16]: DoubleRow lhsT outer free stride must be
                # 16B-aligned; only column 0 is consumed
                ones2 = small.tile([128, 2, 16], F8, tag="ones2")
                nc.gpsimd.dma_start(ones2[:], ones2_ap[:])
                bo4 = small.tile([128, CI], F32, tag="bo4")
                for t in range(CI):
                    nc.gpsimd.dma_start(bo4[:, t:t + 1],
                                        bo_ap2[t * 128:(t + 1) * 128, :])

                eps_t = small.tile([128, 4], F32, tag="eps")
                nc.vector.memset(eps_t[:], EPS)
                noff = small.tile([128, 1], F32, tag="noff")
                nc.vector.memset(noff[:], -EXP_OFF)
                ga4 = small.tile([128, CI], F32, tag="ga4")
                gb4 = small.tile([128, CI], F32, tag="gb4")
                bq4 = small.tile([128, CI], F32, tag="bq4")
                for t in range(CI):
                    nc.gpsimd.dma_start(ga4[:, t:t + 1], ga_ap[t * 128:(t + 1) * 128, :])
                    nc.gpsimd.dma_start(gb4[:, t:t + 1], gb_ap[t * 128:(t + 1) * 128, :])
                    nc.gpsimd.dma_start(bq4[:, t:t + 1], bq_ap[t * 128:(t + 1) * 128, :])

                # weights on the vector DGE ring: don't queue behind x
                w_sb = {}
                for w in ("wqT", "wkT", "wvT", "woT"):
                    w_sb[w] = []
                    for p in range(CP):
                        tt = wsb.tile([128, 2, C], F8, tag=f"{w}{p}",
                                      name=f"{w}{p}")
                        nc.scalar.dma_start(tt[:],
                                            w_aps[w][p * 128:(p + 1) * 128, :])
                        w_sb[w].append(tt)

                # stats per quarter: DVE row-sum, ACT square+accum
                # col layout: 4t+qq -> sum, 16+4t+qq -> sumsq
                stats = small.tile([128, 8 * CI], F32, tag="stats")
                for t in range(CI):
                    for qq in range(4):
                        sl = x_t[t][:, qq * Q4:(qq + 1) * Q4]
                        nc.vector.reduce_sum(
                            stats[:, 4 * t + qq:4 * t + qq + 1],
                            sl, axis=mybir.AxisListType.X)
                        scr = scrap_pool.tile([128, Q4], BF16, tag="scrap")
                        nc.scalar.activation(
                            scr[:], sl, AF.Square,
                            accum_out=stats[:, 16 + 4 * t + qq:17 + 4 * t + qq])

                # group-merge across partitions (replicated per-partition)
                G = gn_ps.tile([128, 8 * CI], F32, tag="G")
                nc.tensor.matmul(G[:], selBB[:], stats[:], start=True, stop=True)

                # selBB carries 1/(16*HW), so G is already mean-scaled
                Gs = small.tile([128, 8 * CI], F32, tag="Gs")
                nc.vector.tensor_copy(Gs[:], G[:])
                # pairwise-add quarters twice: 32 cols -> 16 -> 8
                p16 = small.tile([128, 4 * CI], F32, tag="p16")
                nc.vector.tensor_tensor(p16[:], Gs[:, 0:32:2], Gs[:, 1:32:2],
                                        op=ALU.add)
                mean8 = small.tile([128, 2 * CI], F32, tag="mean8")
                nc.vector.tensor_tensor(mean8[:], p16[:, 0:16:2], p16[:, 1:16:2],
                                        op=ALU.add)
                mean4 = mean8[:, 0:CI]
                ex24 = mean8[:, CI:2 * CI]
                m24 = small.tile([128, CI], F32, tag="m24")
                nc.vector.tensor_tensor(m24[:], mean4, mean4, op=ALU.mult)
                var4 = small.tile([128, CI], F32, tag="var4")
                nc.vector.tensor_tensor(var4[:], ex24, m24[:], op=ALU.subtract)
                sd4 = small.tile([128, CI], F32, tag="sd4")
                nc.scalar.activation(sd4[:], var4[:], AF.Sqrt, bias=eps_t[:, 0:1])
                rstd4 = small.tile([128, CI], F32, tag="rstd4")
                nc.vector.reciprocal(rstd4[:], sd4[:])
                a4 = small.tile([128, CI], F32, tag="a4")
                nc.vector.tensor_tensor(a4[:], rstd4[:], ga4[:], op=ALU.mult)
                am4 = small.tile([128, CI], F32, tag="am4")
                nc.vector.tensor_tensor(am4[:], mean4, a4[:], op=ALU.mult)
                b4 = small.tile([128, CI], F32, tag="b4")
                nc.vector.tensor_tensor(b4[:], gb4[:], am4[:], op=ALU.subtract)

                # normalize into fp8 pair layout: DVE first half, ACT second
                for t in range(CI):
                    dst = hn[t // 2][:, (t % 2):(t % 2) + 1, :]
                    nc.vector.tensor_scalar(dst[:, :, 0:H2], x_t[t][:, 0:H2],
                                            a4[:, t:t + 1], b4[:, t:t + 1],
                                            ALU.mult, ALU.add)
                    nc.scalar.activation(dst[:, :, H2:HW], x_t[t][:, H2:HW],
                                         AF.Identity, bias=b4[:, t:t + 1],
                                         scale=a4[:, t:t + 1])

            # ================= Phase 2: projections (fp8 DoubleRow) ========
            _kqv_cm = tc.tile_pool(name="kqv", bufs=1)
            kqv = _kqv_cm.__enter__()
            k_pair = [kqv.tile([128, 2, HW], F8, tag=f"k{p}", name=f"k{p}")
                      for p in range(CP)]
            q_pair = [kqv.tile([128, 2, NQ], F8, tag=f"q{p}", name=f"q{p}")
                      for p in range(CP)]
            vT_pair = [kqv.tile([128, 2, C], F8, tag=f"vT{jp}", name=f"vT{jp}")
                       for jp in range(JP)]

            with tc.tile_pool(name="proj_ps", bufs=4, space="PSUM") as proj_ps:
                # k = wkT.T @ hn; per co, 4 psum pair-tiles, pair-evac'd
                for co in range(CI):
                    pps = [proj_ps.tile([128, 1024], F32, tag="proj",
                                        name=f"kps{co}_{g}") for g in range(4)]
                    for p in range(CP):
                        for jb in range(JB):
                            nc.tensor.matmul(
                                pps[jb // 2][:, (jb % 2) * 512:(jb % 2 + 1) * 512],
                                w_sb["wkT"][p][:, :, co * 128:(co + 1) * 128],
                                hn[p][:, :, jb * 512:(jb + 1) * 512],
                                start=(p == 0), stop=(p == CP - 1),
                                perf_mode=DR)
                    for g in range(4):
                        dst = k_pair[co // 2][:, (co % 2):(co % 2) + 1,
                                              g * 1024:(g + 1) * 1024]
                        if g % 2 == 0:
                            nc.vector.tensor_copy(dst, pps[g][:])
                        else:
                            nc.scalar.activation(dst, pps[g][:], AF.Copy)
                # q = wqT.T @ hn[:, :NQ] + bq
                for co in range(CI):
                    pps = [proj_ps.tile([128, 1024], F32, tag="proj",
                                        name=f"qps{co}_{g}") for g in range(2)]
                    for p in range(CP):
                        for ib in range(IB):
                            nc.tensor.matmul(
                                pps[ib // 2][:, (ib % 2) * 512:(ib % 2 + 1) * 512],
                                w_sb["wqT"][p][:, :, co * 128:(co + 1) * 128],
                                hn[p][:, :, ib * 512:(ib + 1) * 512],
                                start=(p == 0), stop=(p == CP - 1),
                                perf_mode=DR)
                    for g in range(2):
                        dst = q_pair[co // 2][:, (co % 2):(co % 2) + 1,
                                              g * 1024:(g + 1) * 1024]
                        nc.vector.tensor_scalar(dst, pps[g][:],
                                                bq4[:, co:co + 1], None,
                                                ALU.add)
                # vT[j, c] = hn_chunk.T @ wvT (bias folded into bo2)
                for jp in range(JP):
                    ps = proj_ps.tile([128, 1024], F32, tag="proj",
                                      name=f"vps{jp}")
                    for m in range(2):
                        jc = 2 * jp + m
                        for p in range(CP):
                            nc.tensor.matmul(
                                ps[:, m * 512:(m + 1) * 512],
                                hn[p][:, :, jc * 128:(jc + 1) * 128],
                                w_sb["wvT"][p][:],
                                start=(p == 0), stop=(p == CP - 1),
                                perf_mode=DR)
                    if jp % 2 == 0:
                        nc.vector.tensor_copy(vT_pair[jp][:], ps[:])
                    else:
                        nc.scalar.activation(vT_pair[jp][:], ps[:], AF.Copy)

            # ================= Phase 3: attention + output =================
            # Software-pipelined: scores+exp of pair-step s+1 are emitted
            # before the attn@V of step s, so the PE never waits on ACT exp.
            with (
                tc.tile_pool(name="sc_ps", bufs=3, space="PSUM") as sc_ps,
                tc.tile_pool(name="ao_ps", bufs=1, space="PSUM") as ao_ps,
            ):
                seq = [(ib, jp) for ib in range(IB) for jp in range(JP)]
                at_tiles = {}
                ao_cur = {}
                den_cur = {}
                xres_cur = {}

                def emit_scores(step):
                    ib, jp = seq[step]
                    at = attn_pool.tile([128, 2, 512], F8, tag="at",
                                        name=f"at{ib}_{jp}")
                    for m in range(2):
                        jc = 2 * jp + m
                        sc = sc_ps.tile([128, 512], F32, tag="sc",
                                        name=f"sc{ib}_{jc}")
                        for p in range(CP):
                            nc.tensor.matmul(
                                sc[:],
                                k_pair[p][:, :, jc * 128:(jc + 1) * 128],
                                q_pair[p][:, :, ib * 512:(ib + 1) * 512],
                                start=(p == 0), stop=(p == CP - 1),
                                perf_mode=DR)
                        nc.scalar.activation(at[:, m:m + 1, :], sc[:], AF.Exp,
                                             bias=noff[:], scale=SCALE)
                    at_tiles[step] = at

                emit_scores(0)
                emit_scores(1)
                for step, (ib, jp) in enumerate(seq):
                    if jp == 0:
                        # i-block entry: residual prefetch + fresh accumulators
                        xres_cur[ib] = []
                        for co in range(CI):
                            xr = xres_pool.tile([128, 512], F32, tag=f"xres{co}",
                                                name=f"xres{ib}_{co}")
                            nc.scalar.dma_start(
                                xr[:],
                                xres_ap[co * 128:(co + 1) * 128,
                                        ib * 512:(ib + 1) * 512])
                            xr2 = xres_pool.tile([128, 512], F32, tag=f"xrb{co}",
                                                 name=f"xrb{ib}_{co}")
                            nc.gpsimd.tensor_scalar(xr2[:], xr[:],
                                                    bo4[:, co:co + 1], None,
                                                    ALU.add)
                            xres_cur[ib].append(xr2)
                        ao_cur[ib] = [ao_ps.tile([128, 512], F32, tag=f"ao{cc}",
                                                 name=f"ao{ib}_{cc}")
                                      for cc in range(CI)]
                        den_cur[ib] = ao_ps.tile([1, 512], F32, tag="den",
                                                 name=f"den{ib}")
                    if step + 2 < len(seq):
                        emit_scores(step + 2)
                    at = at_tiles.pop(step)
                    ao = ao_cur[ib]
                    m_last = None
                    for cc in range(CI):
                        m_last = nc.tensor.matmul(
                            ao[cc][:],
                            vT_pair[jp][:, :, cc * 128:(cc + 1) * 128],
                            at[:],
                            start=(jp == 0), stop=(jp == JP - 1),
                            perf_mode=DR)
                    m_den = nc.tensor.matmul(den_cur[ib][:], ones2[:, :, 0:1],
                                             at[:],
                                             start=(jp == 0), stop=(jp == JP - 1),
                                             perf_mode=DR)
                    tile.add_dep_helper(m_last.ins, m_den.ins, sync=False,
                                        reason="keep den after ao group")
                    if jp == JP - 1:
                        # post block: recip+broadcast, normalized fp8 evac,
                        # o-proj, +bo2, residual
                        den = den_cur[ib]
                        recb = dsb_pool.tile([1, 512], F32, tag="recb",
                                             name=f"recb{ib}")
                        nc.vector.reciprocal_approx_fast(recb[:], den[:])
                        rb = rb_pool.tile([128, 512], F32, tag="rb",
                                          name=f"rb{ib}")
                        nc.gpsimd.partition_broadcast(rb[:], recb[:])
                        ao_n = []
                        for pp in range(CP):
                            an = aosb_pool.tile([128, 2, 512], F8, tag=f"aon{pp}",
                                                name=f"aon{ib}_{pp}")
                            for m in range(2):
                                cc = 2 * pp + m
                                # normalize during evac: fp8 only ever sees
                                # softmax-normalized values (|.| small)
                                nc.vector.tensor_tensor(an[:, m:m + 1, :],
                                                        ao[cc][:], rb[:],
                                                        op=ALU.mult)
                            ao_n.append(an)
                        for co in range(CI):
                            yp = sc_ps.tile([128, 512], F32, tag="sc",
                                            name=f"y{ib}_{co}")
                            for pp in range(CP):
                                nc.tensor.matmul(
                                    yp[:],
                                    w_sb["woT"][pp][:, :, co * 128:(co + 1) * 128],
                                    ao_n[pp][:],
                                    start=(pp == 0), stop=(pp == CP - 1),
                                    perf_mode=DR)
                            ot = oevac.tile([128, 512], F32, tag="ot")
                            nc.vector.tensor_tensor(ot[:], yp[:],
                                                    xres_cur[ib][co][:],
                                                    op=ALU.add)
                            nc.sync.dma_start(
                                out_ap[co * 128:(co + 1) * 128,
                                       ib * 512:(ib + 1) * 512],
                                ot[:])
            _kqv_cm.__exit__(None, None, None)

    nc.compile()
    return nc


def _prep_inputs(x, norm_scale, norm_bias, wq, bq, wk, bk, wv, bv, wo, bo):
    bf16 = ml_dtypes.bfloat16
    f8 = ml_dtypes.float8_e4m3
    f32 = np.float32
    x = np.asarray(x, f32).reshape(B, C, HW)

    def w8pair(w):
        # w: [out, in] f32 -> wT fp8 pair layout [2*128, 2*512]:
        # rows p*128..(p+1)*128 hold [128, 2, 512] = chunks (2p, 2p+1)
        wT = np.ascontiguousarray(np.asarray(w, f32).T)  # [cin, cout]
        wT8 = np.clip(wT, -240, 240).astype(f8)
        out = np.empty((2 * 128, 2 * C), f8)
        for p in range(CP):
            a = wT8[256 * p:256 * p + 128]          # [128, 512]
            b = wT8[256 * p + 128:256 * (p + 1)]    # [128, 512]
            out[p * 128:(p + 1) * 128] = np.stack([a, b], axis=1).reshape(128, 2 * C)
        return out

    common = {
        "wqT": w8pair(wq),
        "wkT": w8pair(wk),
        "wvT": w8pair(wv),
        "woT": w8pair(wo),
        "bq2": np.asarray(bq, f32).reshape(C, 1),
        "bo2c": (np.asarray(wo, f32) @ np.asarray(bv, f32)
                 + np.asarray(bo, f32)).reshape(C, 1),
        "ga": np.asarray(norm_scale, f32).reshape(C, 1),
        "gb": np.asarray(norm_bias, f32).reshape(C, 1),
        "selBB": (np.arange(128)[:, None] // 16
                  == np.arange(128)[None, :] // 16).astype(f32)
                 * np.float32(1.0 / (16 * HW)),
        "ones2": np.ones((128, 32), f8),
    }
    in_maps = []
    for c in range(N_CORES):
        b, h = divmod(c, 2)
        mine = x[b][:, h * NQ:(h + 1) * NQ]
        other = x[b][:, (1 - h) * NQ:(2 - h) * NQ]
        xf = np.ascontiguousarray(np.concatenate([mine, other], axis=1))
        in_maps.append({"xbf": xf.astype(bf16),
                        "xres": np.ascontiguousarray(mine),
                        **common})
    return in_maps


def _run(in_maps, **kwargs):
    from concourse.bass_utils import run_bass_kernel_spmd
    if "nc" not in _cache:
        _cache["nc"] = _build()
    return run_bass_kernel_spmd(_cache["nc"], in_maps,
                                core_ids=list(range(N_CORES)), **kwargs)


def kernel(x, norm_scale, norm_bias, wq, bq, wk, bk, wv, bv, wo, bo):
    in_maps = _prep_inputs(x, norm_scale, norm_bias, wq, bq, wk, bk, wv, bv,
                           wo, bo)
    res = _run(in_maps)
    out = np.empty((B, C, HW), np.float32)
    for c in range(N_CORES):
        b, h = divmod(c, 2)
        out[b][:, h * NQ:(h + 1) * NQ] = res.results[c]["out"]
    return out.reshape(B, C, 64, 64)
